# revision 8
# baseline (speedup 1.0000x reference)
"""GCN 3-layer (improved GCNConv + BN + ReLU) Bass/Tile kernel for 8 trn2 NeuronCores.

Strategy (graph/data parallel, per sharding hint):
  - Nodes partitioned into 8 contiguous ranges (6250/core); edges routed to the
    core owning their destination node (host-side routing = the sharding step).
  - Per layer: each core computes u = dinv[n] * (h @ W) for its nodes (PE),
    AllGathers u (bf16, [N,128]) into HBM, then per 128-edge tile:
      * dma_gather pulls u[src] rows into SBUF (messages, bf16)
      * DVE builds a one-hot matrix S[e, dst_slot] = (localdst[e] == iota) (bf16)
      * PE accumulates psum[feat, dst_slot] += msg.T @ S over the ~35 tiles of a
        128-destination window; the self-loop term is folded in as one extra
        matmul with lhsT=u_rows[window], rhs=2*I.
    The int16 gather-index limit is handled by splitting the u table at row
    32768 into two gather sources ("halves").
  - agg *= dinv[dst] (broadcast tile), then BN stats (free-dim reduce +
    AllReduce [128,2]) and fused scale/shift+ReLU on ScalarE.
  - h lives as [128 feat, nodes] f32 in SBUF the whole time; output is
    PE-transposed back to [nodes, 128].

All per-core program structure (tile counts per window/half) is padded to the
max across cores so one SPMD program serves all 8 cores; pads use gather row 0
and localdst=200 (one-hot row of zeros -> no contribution).
"""

import numpy as np
import ml_dtypes

import concourse.bass as bass
import concourse.bacc as bacc
import concourse.tile as tile
from concourse import bass_utils, mybir

F32 = mybir.dt.float32
BF16 = mybir.dt.bfloat16
I16 = mybir.dt.int16
BF16_NP = ml_dtypes.bfloat16

BN_EPS = 1e-5
PAD_LD = 200.0  # localdst value for pad slots; > 127 so one-hot row is all zero


class Cfg:
    def __init__(self, n_nodes=50000, n_edges=1600000, n_cores=8, chunk_tiles=8,
                 half=32768):
        self.N = n_nodes
        self.E = n_edges
        self.NC = n_cores
        self.D = 128
        assert n_nodes % n_cores == 0
        self.NPC = n_nodes // n_cores          # nodes per core
        self.W = (self.NPC + 127) // 128       # windows (= node blocks) per core
        self.NB = self.W                       # alias
        self.NPAD = self.W * 128               # padded nodes per core
        self.HALF = half                       # u-table split row (int16 idx limit)
        self.CT = chunk_tiles                  # gather chunk size in 128-edge tiles


def preprocess(cfg, x, edge_index, Ws, gs, bes):
    """Host-side edge routing/sharding. Returns (in_maps, meta)."""
    c = cfg
    src = np.asarray(edge_index[0]).astype(np.int64)
    dst = np.asarray(edge_index[1]).astype(np.int64)
    x = np.asarray(x, dtype=np.float32)

    deg = (np.bincount(dst, minlength=c.N) + 2.0).astype(np.float32)
    dinv = (1.0 / np.sqrt(deg)).astype(np.float32)

    core = dst // c.NPC
    wloc = (dst % c.NPC) // 128
    half = (src >= c.HALF).astype(np.int64)

    gkey = (core * c.W + wloc) * 2 + half
    cnt = np.bincount(gkey, minlength=c.NC * c.W * 2).reshape(c.NC, c.W, 2)
    # tiles per (window, half) group -- shared across cores (max + ceil)
    nt = np.ceil(cnt.max(axis=0) / 128.0).astype(np.int64)  # [W, 2]
    nt = np.maximum(nt, 1)
    s_tiles = nt.sum(axis=0)  # [2] tiles per half-stream
    off_tiles = np.cumsum(nt, axis=0) - nt  # [W, 2] tile offset within half-stream
    ntw = nt[:, 0] + nt[:, 1]
    ldcol0 = np.cumsum(ntw) - ntw           # [W] first localdst column of window
    NT = int(ntw.sum())

    meta = dict(nt=nt, s_tiles=s_tiles, off_tiles=off_tiles, ldcol0=ldcol0, NT=NT)

    # constants (identical for every core)
    iota_b = np.tile(np.arange(128, dtype=np.float32)[None, :], (128, 1)).astype(BF16_NP)
    eye2_b = (2.0 * np.eye(128, dtype=np.float32)).astype(BF16_NP)
    id_f = np.eye(128, dtype=np.float32)

    in_maps = []
    for ci in range(c.NC):
        base = ci * c.NPC
        m = core == ci
        s_c = src[m]
        d_c = dst[m]
        wv = wloc[m]
        hv = half[m]
        gk = wv * 2 + hv
        order = np.argsort(gk, kind="stable")
        s_c, d_c, wv, hv, gk = s_c[order], d_c[order], wv[order], hv[order], gk[order]
        cnt_c = np.bincount(gk, minlength=c.W * 2)
        gstart = np.cumsum(cnt_c) - cnt_c
        rank = np.arange(s_c.size) - gstart[gk]

        idx_flat = [np.zeros(int(s_tiles[0]) * 128, np.int16),
                    np.zeros(int(s_tiles[1]) * 128, np.int16)]
        pos_slot = off_tiles[wv, hv] * 128 + rank
        for h in (0, 1):
            mh = hv == h
            idx_flat[h][pos_slot[mh]] = (s_c[mh] - h * c.HALF).astype(np.int16)

        ld_flat = np.full(meta["NT"] * 128, PAD_LD, np.float32)
        ldpos = (ldcol0[wv] + hv * nt[wv, 0] + rank // 128) * 128 + rank % 128
        ld_flat[ldpos] = (d_c - (base + wv * 128)).astype(np.float32)

        def wrap16(a):
            t = a.reshape(-1, 16).T  # [16, S/16], element (p, j) = a[16j+p]
            return np.ascontiguousarray(np.tile(t, (8, 1)))

        dv = np.zeros(c.NPAD, np.float32)
        dv[:c.NPC] = dinv[base:base + c.NPC]
        xt = np.zeros((128, c.NPAD), np.float32)
        xt[:, :c.NPC] = x[base:base + c.NPC].T

        im = {
            "x_t": xt,
            "idx0": wrap16(idx_flat[0]),
            "idx1": wrap16(idx_flat[1]),
            "ld": np.ascontiguousarray(ld_flat.reshape(meta["NT"], 128).T),
            "dinv_col": np.ascontiguousarray(dv.reshape(c.NB, 128).T),
            "dinv_b": np.ascontiguousarray(np.tile(dv[None, :], (128, 1))),
            "iota_b": iota_b,
            "eye2_b": eye2_b,
            "id_f": id_f,
        }
        for li in range(3):
            im[f"w{li}"] = np.asarray(Ws[li], np.float32)
            im[f"g{li}"] = np.asarray(gs[li], np.float32).reshape(128, 1)
            im[f"be{li}"] = np.asarray(bes[li], np.float32).reshape(128, 1)
        in_maps.append(im)
    return in_maps, meta


def build(cfg, meta, stages=("u", "ag", "gather", "agg", "bn", "out")):
    stages = set(stages)
    c = cfg
    nt = meta["nt"]
    s_tiles = meta["s_tiles"]
    NT = meta["NT"]

    nc = bacc.Bacc("TRN2", target_bir_lowering=False, debug=False, num_devices=c.NC)

    x_t = nc.dram_tensor("x_t", [128, c.NPAD], F32, kind="ExternalInput")
    idx_d = [nc.dram_tensor(f"idx{h}", [128, int(s_tiles[h]) * 8], I16, kind="ExternalInput")
             for h in (0, 1)]
    ld_d = nc.dram_tensor("ld", [128, NT], F32, kind="ExternalInput")
    dinv_col_d = nc.dram_tensor("dinv_col", [128, c.NB], F32, kind="ExternalInput")
    dinv_b_d = nc.dram_tensor("dinv_b", [128, c.NPAD], F32, kind="ExternalInput")
    iota_d = nc.dram_tensor("iota_b", [128, 128], BF16, kind="ExternalInput")
    eye2_d = nc.dram_tensor("eye2_b", [128, 128], BF16, kind="ExternalInput")
    id_d = nc.dram_tensor("id_f", [128, 128], F32, kind="ExternalInput")
    w_d = [nc.dram_tensor(f"w{l}", [128, 128], F32, kind="ExternalInput") for l in range(3)]
    g_d = [nc.dram_tensor(f"g{l}", [128, 1], F32, kind="ExternalInput") for l in range(3)]
    be_d = [nc.dram_tensor(f"be{l}", [128, 1], F32, kind="ExternalInput") for l in range(3)]

    y_d = nc.dram_tensor("y", [c.NPAD, 128], F32, kind="ExternalOutput")

    shared = "Shared" if c.NC > 4 else "Local"
    u_loc = nc.dram_tensor("u_loc", [c.NPAD, 128], BF16, kind="Internal")
    u_full = nc.dram_tensor("u_full", [c.N, 128], BF16, kind="Internal", addr_space=shared)
    stats_in = nc.dram_tensor("stats_in", [128, 2], F32, kind="Internal")
    stats_out = nc.dram_tensor("stats_out", [128, 2], F32, kind="Internal", addr_space=shared)

    rg = [list(range(c.NC))]

    # gather chunk layout per half: list of (tile_start, n_tiles)
    chunks = []
    for h in (0, 1):
        S = int(s_tiles[h])
        ch = []
        t0 = 0
        while t0 < S:
            ch.append((t0, min(c.CT, S - t0)))
            t0 += min(c.CT, S - t0)
        chunks.append(ch)

    with tile.TileContext(nc) as tc:
        with (
            tc.tile_pool(name="persist", bufs=1) as pp,
            tc.tile_pool(name="msg0", bufs=3) as m0p,
            tc.tile_pool(name="msg1", bufs=3) as m1p,
            tc.tile_pool(name="onehot", bufs=8) as sp,
            tc.tile_pool(name="cols", bufs=1) as cp,
            tc.tile_pool(name="pw", bufs=4, space="PSUM") as pwp,
            tc.tile_pool(name="pu", bufs=2, space="PSUM") as pup,
        ):
            # ---- persistent SBUF ----
            h_sb = pp.tile([128, c.NPAD], F32, tag="h")
            u_rows = pp.tile([128, c.NPAD], BF16, tag="u_rows")
            agg = pp.tile([128, c.NPAD], F32, tag="agg")
            dinv_b = pp.tile([128, c.NPAD], F32, tag="dinv_b")
            ld_sb = pp.tile([128, NT], F32, tag="ld")
            idx_sb = [pp.tile([128, int(s_tiles[h]) * 8], I16, tag=f"idx{h}", name=f"idx{h}_sb") for h in (0, 1)]
            dinv_col = pp.tile([128, c.NB], F32, tag="dinv_col")
            iota_sb = pp.tile([128, 128], BF16, tag="iota")
            eye2_sb = pp.tile([128, 128], BF16, tag="eye2")
            id_sb = pp.tile([128, 128], F32, tag="idf")
            w_sb = [pp.tile([128, 128], F32, tag=f"w{l}", name=f"w{l}_sb") for l in range(3)]
            g_sb = [pp.tile([128, 1], F32, tag=f"g{l}", name=f"g{l}_sb") for l in range(3)]
            be_sb = [pp.tile([128, 1], F32, tag=f"be{l}", name=f"be{l}_sb") for l in range(3)]

            nc.sync.dma_start(h_sb[:], x_t.ap())
            nc.sync.dma_start(ld_sb[:], ld_d.ap())
            for h in (0, 1):
                nc.sync.dma_start(idx_sb[h][:], idx_d[h].ap())
            nc.sync.dma_start(dinv_col[:], dinv_col_d.ap())
            nc.sync.dma_start(dinv_b[:], dinv_b_d.ap())
            nc.sync.dma_start(iota_sb[:], iota_d.ap())
            nc.sync.dma_start(eye2_sb[:], eye2_d.ap())
            nc.sync.dma_start(id_sb[:], id_d.ap())
            for l in range(3):
                nc.sync.dma_start(w_sb[l][:], w_d[l].ap())
                nc.sync.dma_start(g_sb[l][:], g_d[l].ap())
                nc.sync.dma_start(be_sb[l][:], be_d[l].ap())

            u_loc_ap = u_loc.ap().rearrange("(g p) f -> p g f", p=128)

            for l in range(3):
                # ---- u_rows = dinv[n] * (h @ W) in [node, feat] layout ----
                for g in range(c.NB):
                    pu = pup.tile([128, 128], F32, tag="pu")
                    nc.tensor.matmul(pu[:], lhsT=h_sb[:, g * 128:(g + 1) * 128],
                                     rhs=w_sb[l][:], start=True, stop=True)
                    nc.vector.tensor_scalar(u_rows[:, g * 128:(g + 1) * 128], pu[:],
                                            dinv_col[:, g:g + 1], None,
                                            mybir.AluOpType.mult)
                nc.sync.dma_start(u_loc_ap, u_rows[:].rearrange("p (g f) -> p g f", f=128))

                if "ag" in stages:
                    nc.gpsimd.collective_compute(
                        "AllGather", mybir.AluOpType.bypass, replica_groups=rg,
                        ins=[u_loc.ap()[0:c.NPC, :]], outs=[u_full.ap()],
                    )

                # ---- gather message chunks ----
                msg_tiles = [[], []]
                for h in (0, 1) if "gather" in stages else ():
                    src_ap = u_full.ap()[0:c.HALF, :] if h == 0 else u_full.ap()[c.HALF:c.N, :]
                    pool = m0p if h == 0 else m1p
                    for (t0, ctn) in chunks[h]:
                        mt = pool.tile([128, c.CT, 128], BF16, tag=f"m{h}", name=f"m{h}_chunk")
                        nidx = ctn * 128
                        nc.gpsimd.dma_gather(
                            out_ap=mt[:, 0:ctn, :],
                            in_ap=src_ap,
                            idxs_ap=idx_sb[h][:, t0 * 8:(t0 + ctn) * 8],
                            num_idxs=nidx,
                            num_idxs_reg=nidx,
                            elem_size=128,
                        )
                        msg_tiles[h].append((t0, ctn, mt))

                def msg_slice(h, gtile):
                    for (t0, ctn, mt) in msg_tiles[h]:
                        if t0 <= gtile < t0 + ctn:
                            return mt[:, gtile - t0, :]
                    raise AssertionError

                # ---- windows: one-hot + matmul accumulate ----
                tcol = 0
                for w in range(c.W if "agg" in stages else 0):
                    pw = pwp.tile([128, 128], F32, tag="pw")
                    first = True
                    for h in (0, 1):
                        for k in range(int(nt[w, h])):
                            gtile = int(meta["off_tiles"][w, h]) + k
                            s_t = sp.tile([128, 128], BF16, tag="s")
                            nc.vector.tensor_scalar(s_t[:], iota_sb[:],
                                                    ld_sb[:, tcol:tcol + 1], None,
                                                    mybir.AluOpType.is_equal)
                            nc.tensor.matmul(pw[:], lhsT=msg_slice(h, gtile), rhs=s_t[:],
                                             start=first, stop=False)
                            first = False
                            tcol += 1
                    nc.tensor.matmul(pw[:], lhsT=u_rows[:, w * 128:(w + 1) * 128],
                                     rhs=eye2_sb[:], start=first, stop=True)
                    wd = min(128, c.NPC - w * 128)
                    nc.vector.tensor_mul(agg[:, w * 128:w * 128 + wd], pw[:, 0:wd],
                                         dinv_b[:, w * 128:w * 128 + wd])

                if "agg" not in stages:
                    nc.vector.memset(agg[:], 0.01 * (l + 1))
                if "bn" not in stages:
                    if "bn1" in stages:
                        sum_c = cp.tile([128, 1], F32, tag="sum")
                        ssq_c = cp.tile([128, 1], F32, tag="ssq")
                        nc.vector.tensor_reduce(sum_c[:], agg[:, 0:c.NPC],
                                                axis=mybir.AxisListType.X, op=mybir.AluOpType.add)
                        if "ttr" in stages:
                            nc.vector.tensor_tensor_reduce(
                                h_sb[:, 0:c.NPC], agg[:, 0:c.NPC], agg[:, 0:c.NPC], 1.0, 0.0,
                                mybir.AluOpType.mult, mybir.AluOpType.add, ssq_c[:])
                        else:
                            nc.vector.tensor_reduce(ssq_c[:], agg[:, 0:c.NPC],
                                                    axis=mybir.AxisListType.X, op=mybir.AluOpType.add)
                        st_sb = cp.tile([128, 2], F32, tag="st")
                        nc.vector.tensor_copy(st_sb[:, 0:1], sum_c[:])
                        nc.vector.tensor_copy(st_sb[:, 1:2], ssq_c[:])
                        nc.sync.dma_start(stats_in.ap(), st_sb[:])
                        nc.gpsimd.collective_compute(
                            "AllReduce", mybir.AluOpType.add, replica_groups=rg,
                            ins=[stats_in.ap()], outs=[stats_out.ap()],
                        )
                        st2 = cp.tile([128, 2], F32, tag="st2")
                        nc.sync.dma_start(st2[:], stats_out.ap())
                    nc.scalar.activation(h_sb[:, 0:c.NPC], agg[:, 0:c.NPC],
                                         mybir.ActivationFunctionType.Relu)
                    continue
                # ---- BatchNorm (training stats over all N nodes) + ReLU ----
                sum_c = cp.tile([128, 1], F32, tag="sum")
                ssq_c = cp.tile([128, 1], F32, tag="ssq")
                nc.vector.tensor_reduce(sum_c[:], agg[:, 0:c.NPC],
                                        axis=mybir.AxisListType.X, op=mybir.AluOpType.add)
                nc.vector.tensor_mul(h_sb[:, 0:c.NPC], agg[:, 0:c.NPC], agg[:, 0:c.NPC])
                nc.vector.tensor_reduce(ssq_c[:], h_sb[:, 0:c.NPC],
                                        axis=mybir.AxisListType.X, op=mybir.AluOpType.add)
                st_sb = cp.tile([128, 2], F32, tag="st")
                nc.vector.tensor_copy(st_sb[:, 0:1], sum_c[:])
                nc.vector.tensor_copy(st_sb[:, 1:2], ssq_c[:])
                nc.sync.dma_start(stats_in.ap(), st_sb[:])
                nc.gpsimd.collective_compute(
                    "AllReduce", mybir.AluOpType.add, replica_groups=rg,
                    ins=[stats_in.ap()], outs=[stats_out.ap()],
                )
                st2 = cp.tile([128, 2], F32, tag="st2")
                nc.sync.dma_start(st2[:], stats_out.ap())
                mean_c = cp.tile([128, 1], F32, tag="mean")
                var_c = cp.tile([128, 1], F32, tag="var")
                t1 = cp.tile([128, 1], F32, tag="t1")
                s_col = cp.tile([128, 1], F32, tag="scol")
                t_col = cp.tile([128, 1], F32, tag="tcol")
                inv_n = 1.0 / float(c.N)
                nc.vector.tensor_scalar_mul(mean_c[:], st2[:, 0:1], inv_n)
                nc.vector.tensor_scalar_mul(var_c[:], st2[:, 1:2], inv_n)   # E[x^2]
                nc.vector.tensor_mul(t1[:], mean_c[:], mean_c[:])
                nc.vector.tensor_sub(var_c[:], var_c[:], t1[:])
                nc.vector.tensor_scalar_add(var_c[:], var_c[:], BN_EPS)
                nc.scalar.sqrt(t1[:], var_c[:])
                nc.vector.reciprocal(t1[:], t1[:])                           # rstd
                nc.vector.tensor_mul(s_col[:], g_sb[l][:], t1[:])
                nc.vector.tensor_mul(t_col[:], mean_c[:], s_col[:])
                nc.vector.tensor_sub(t_col[:], be_sb[l][:], t_col[:])
                # h = relu(agg * s + t)
                nc.scalar.activation(h_sb[:, 0:c.NPC], agg[:, 0:c.NPC],
                                     mybir.ActivationFunctionType.Relu,
                                     bias=t_col[:], scale=s_col[:])

            # ---- output: transpose h -> [node, feat] and store ----
            for g in range(c.NB if "out" in stages else 0):
                pt = pup.tile([128, 128], F32, tag="pu")
                nc.tensor.transpose(pt[:], h_sb[:, g * 128:(g + 1) * 128], id_sb[:])
                nc.vector.tensor_copy(agg[:, g * 128:(g + 1) * 128], pt[:])
            nc.sync.dma_start(y_d.ap().rearrange("(g p) f -> p g f", p=128),
                              agg[:].rearrange("p (g f) -> p g f", f=128))

    nc.compile()
    return nc


_CACHE = {}


def _get_program(cfg, meta):
    key = (cfg.N, cfg.E, cfg.NC, cfg.CT, meta["NT"],
           tuple(meta["s_tiles"].tolist()), meta["nt"].tobytes())
    if key not in _CACHE:
        _CACHE[key] = build(cfg, meta)
    return _CACHE[key]


def run(cfg, inputs, trace=False):
    Ws = [inputs[f"W{l}"] for l in range(3)]
    gs = [inputs[f"g{l}"] for l in range(3)]
    bes = [inputs[f"be{l}"] for l in range(3)]
    in_maps, meta = preprocess(cfg, inputs["x"], inputs["edge_index"], Ws, gs, bes)
    nc = _get_program(cfg, meta)
    res = bass_utils.run_bass_kernel_spmd(
        nc, in_maps, core_ids=list(range(cfg.NC)), trace=trace)
    out = np.concatenate([res.results[ci]["y"][:cfg.NPC] for ci in range(cfg.NC)], axis=0)
    return out.astype(np.float32), res


def kernel(**inputs):
    cfg = Cfg()
    out, _ = run(cfg, inputs)
    return out


# revision 9
# speedup vs baseline: 1.8759x; 1.8759x over previous
"""GCN 3-layer (improved GCNConv + BN + ReLU) Bass/Tile kernel for 8 trn2 NeuronCores.

Strategy (graph/data parallel, per sharding hint):
  - Nodes partitioned into 8 contiguous ranges (6250/core); edges routed to the
    core owning their destination node (host-side routing = the sharding step).
  - Per layer: each core computes u = dinv[n] * (h @ W) for its nodes (PE),
    AllGathers u (bf16, [N,128]) into HBM, then per 128-edge tile:
      * dma_gather pulls u[src] rows into SBUF (messages, bf16)
      * DVE builds a one-hot matrix S[e, dst_slot] = (localdst[e] == iota) (bf16)
      * PE accumulates psum[feat, dst_slot] += msg.T @ S over the ~35 tiles of a
        128-destination window; the self-loop term is folded in as one extra
        matmul with lhsT=u_rows[window], rhs=2*I.
    The int16 gather-index limit is handled by splitting the u table at row
    32768 into two gather sources ("halves").
  - agg *= dinv[dst] (broadcast tile), then BN stats (free-dim reduce +
    AllReduce [128,2]) and fused scale/shift+ReLU on ScalarE.
  - h lives as [128 feat, nodes] f32 in SBUF the whole time; output is
    PE-transposed back to [nodes, 128].

All per-core program structure (tile counts per window/half) is padded to the
max across cores so one SPMD program serves all 8 cores; pads use gather row 0
and localdst=200 (one-hot row of zeros -> no contribution).
"""

import numpy as np
import ml_dtypes

import concourse.bass as bass
import concourse.bacc as bacc
import concourse.tile as tile
from concourse import bass_utils, mybir

F32 = mybir.dt.float32
BF16 = mybir.dt.bfloat16
I16 = mybir.dt.int16
BF16_NP = ml_dtypes.bfloat16

BN_EPS = 1e-5
PAD_LD = 200.0  # localdst value for pad slots; > 127 so one-hot row is all zero


class Cfg:
    def __init__(self, n_nodes=50000, n_edges=1600000, n_cores=8, chunk_tiles=8,
                 half=32768):
        self.N = n_nodes
        self.E = n_edges
        self.NC = n_cores
        self.D = 128
        assert n_nodes % n_cores == 0
        self.NPC = n_nodes // n_cores          # nodes per core
        self.W = (self.NPC + 127) // 128       # windows (= node blocks) per core
        self.NB = self.W                       # alias
        self.NPAD = self.W * 128               # padded nodes per core
        self.HALF = half                       # u-table split row (int16 idx limit)
        self.CT = chunk_tiles                  # gather chunk size in 128-edge tiles


def preprocess(cfg, x, edge_index, Ws, gs, bes):
    """Host-side edge routing/sharding. Returns (in_maps, meta)."""
    c = cfg
    src = np.asarray(edge_index[0]).astype(np.int64)
    dst = np.asarray(edge_index[1]).astype(np.int64)
    x = np.asarray(x, dtype=np.float32)

    deg = (np.bincount(dst, minlength=c.N) + 2.0).astype(np.float32)
    dinv = (1.0 / np.sqrt(deg)).astype(np.float32)

    core = dst // c.NPC
    wloc = (dst % c.NPC) // 128
    half = (src >= c.HALF).astype(np.int64)

    gkey = (core * c.W + wloc) * 2 + half
    cnt = np.bincount(gkey, minlength=c.NC * c.W * 2).reshape(c.NC, c.W, 2)
    # tiles per (window, half) group -- shared across cores (max + ceil)
    nt = np.ceil(cnt.max(axis=0) / 128.0).astype(np.int64)  # [W, 2]
    nt = np.maximum(nt, 1)
    s_tiles = nt.sum(axis=0)  # [2] tiles per half-stream
    off_tiles = np.cumsum(nt, axis=0) - nt  # [W, 2] tile offset within half-stream
    ntw = nt[:, 0] + nt[:, 1]
    ldcol0 = np.cumsum(ntw) - ntw           # [W] first localdst column of window
    NT = int(ntw.sum())

    meta = dict(nt=nt, s_tiles=s_tiles, off_tiles=off_tiles, ldcol0=ldcol0, NT=NT)

    # constants (identical for every core)
    iota_b = np.tile(np.arange(128, dtype=np.float32)[None, :], (128, 1)).astype(BF16_NP)
    eye2_b = (2.0 * np.eye(128, dtype=np.float32)).astype(BF16_NP)
    id_f = np.eye(128, dtype=np.float32)

    in_maps = []
    for ci in range(c.NC):
        base = ci * c.NPC
        m = core == ci
        s_c = src[m]
        d_c = dst[m]
        wv = wloc[m]
        hv = half[m]
        gk = wv * 2 + hv
        order = np.argsort(gk, kind="stable")
        s_c, d_c, wv, hv, gk = s_c[order], d_c[order], wv[order], hv[order], gk[order]
        cnt_c = np.bincount(gk, minlength=c.W * 2)
        gstart = np.cumsum(cnt_c) - cnt_c
        rank = np.arange(s_c.size) - gstart[gk]

        idx_flat = [np.zeros(int(s_tiles[0]) * 128, np.int16),
                    np.zeros(int(s_tiles[1]) * 128, np.int16)]
        pos_slot = off_tiles[wv, hv] * 128 + rank
        for h in (0, 1):
            mh = hv == h
            idx_flat[h][pos_slot[mh]] = (s_c[mh] - h * c.HALF).astype(np.int16)

        ld_flat = np.full(meta["NT"] * 128, PAD_LD, np.float32)
        ldpos = (ldcol0[wv] + hv * nt[wv, 0] + rank // 128) * 128 + rank % 128
        ld_flat[ldpos] = (d_c - (base + wv * 128)).astype(np.float32)

        def wrap16(a):
            t = a.reshape(-1, 16).T  # [16, S/16], element (p, j) = a[16j+p]
            return np.ascontiguousarray(np.tile(t, (8, 1)))

        dv = np.zeros(c.NPAD, np.float32)
        dv[:c.NPC] = dinv[base:base + c.NPC]
        xt = np.zeros((128, c.NPAD), np.float32)
        xt[:, :c.NPC] = x[base:base + c.NPC].T

        im = {
            "x_t": xt,
            "idx0": wrap16(idx_flat[0]),
            "idx1": wrap16(idx_flat[1]),
            "ld": np.ascontiguousarray(ld_flat.reshape(meta["NT"], 128).T).astype(BF16_NP),
            "dinv_col": np.ascontiguousarray(dv.reshape(c.NB, 128).T),
            "dinv_b": np.ascontiguousarray(np.tile(dv[None, :], (128, 1))),
            "iota_b": iota_b,
            "eye2_b": eye2_b,
            "id_f": id_f,
        }
        for li in range(3):
            im[f"w{li}"] = np.asarray(Ws[li], np.float32)
            im[f"g{li}"] = np.asarray(gs[li], np.float32).reshape(128, 1)
            im[f"be{li}"] = np.asarray(bes[li], np.float32).reshape(128, 1)
        in_maps.append(im)
    return in_maps, meta


def build(cfg, meta, stages=("u", "ag", "gather", "agg", "bn", "out")):
    stages = set(stages)
    c = cfg
    nt = meta["nt"]
    s_tiles = meta["s_tiles"]
    NT = meta["NT"]

    nc = bacc.Bacc("TRN2", target_bir_lowering=False, debug=False, num_devices=c.NC,
                   num_swdge_queues=4)

    x_t = nc.dram_tensor("x_t", [128, c.NPAD], F32, kind="ExternalInput")
    idx_d = [nc.dram_tensor(f"idx{h}", [128, int(s_tiles[h]) * 8], I16, kind="ExternalInput")
             for h in (0, 1)]
    ld_d = nc.dram_tensor("ld", [128, NT], BF16, kind="ExternalInput")
    dinv_col_d = nc.dram_tensor("dinv_col", [128, c.NB], F32, kind="ExternalInput")
    dinv_b_d = nc.dram_tensor("dinv_b", [128, c.NPAD], F32, kind="ExternalInput")
    iota_d = nc.dram_tensor("iota_b", [128, 128], BF16, kind="ExternalInput")
    eye2_d = nc.dram_tensor("eye2_b", [128, 128], BF16, kind="ExternalInput")
    id_d = nc.dram_tensor("id_f", [128, 128], F32, kind="ExternalInput")
    w_d = [nc.dram_tensor(f"w{l}", [128, 128], F32, kind="ExternalInput") for l in range(3)]
    g_d = [nc.dram_tensor(f"g{l}", [128, 1], F32, kind="ExternalInput") for l in range(3)]
    be_d = [nc.dram_tensor(f"be{l}", [128, 1], F32, kind="ExternalInput") for l in range(3)]

    y_d = nc.dram_tensor("y", [c.NPAD, 128], F32, kind="ExternalOutput")

    shared = "Shared" if c.NC > 4 else "Local"
    u_loc = nc.dram_tensor("u_loc", [c.NPAD, 128], BF16, kind="Internal")
    u_full = nc.dram_tensor("u_full", [c.N, 128], BF16, kind="Internal", addr_space=shared)
    stats_in = nc.dram_tensor("stats_in", [128, 2], F32, kind="Internal")
    stats_out = nc.dram_tensor("stats_out", [128, 2], F32, kind="Internal", addr_space=shared)

    rg = [list(range(c.NC))]

    # gather chunk layout per half: list of (tile_start, n_tiles)
    chunks = []
    for h in (0, 1):
        S = int(s_tiles[h])
        ch = []
        t0 = 0
        while t0 < S:
            ch.append((t0, min(c.CT, S - t0)))
            t0 += min(c.CT, S - t0)
        chunks.append(ch)

    with tile.TileContext(nc) as tc:
        with (
            tc.tile_pool(name="persist", bufs=1) as pp,
            tc.tile_pool(name="msg0", bufs=3) as m0p,
            tc.tile_pool(name="msg1", bufs=3) as m1p,
            tc.tile_pool(name="onehot", bufs=8) as sp,
            tc.tile_pool(name="cols", bufs=1) as cp,
            tc.tile_pool(name="pw", bufs=4, space="PSUM") as pwp,
            tc.tile_pool(name="pu", bufs=2, space="PSUM") as pup,
        ):
            # ---- persistent SBUF ----
            h_sb = pp.tile([128, c.NPAD], F32, tag="h")
            u_rows = pp.tile([128, c.NPAD], BF16, tag="u_rows")
            agg = pp.tile([128, c.NPAD], F32, tag="agg")
            dinv_b = pp.tile([128, c.NPAD], F32, tag="dinv_b")
            ld_sb = pp.tile([128, NT], BF16, tag="ld")
            idx_sb = [pp.tile([128, int(s_tiles[h]) * 8], I16, tag=f"idx{h}", name=f"idx{h}_sb") for h in (0, 1)]
            dinv_col = pp.tile([128, c.NB], F32, tag="dinv_col")
            iota_sb = pp.tile([128, 128], BF16, tag="iota")
            eye2_sb = pp.tile([128, 128], BF16, tag="eye2")
            id_sb = pp.tile([128, 128], F32, tag="idf")
            w_sb = [pp.tile([128, 128], F32, tag=f"w{l}", name=f"w{l}_sb") for l in range(3)]
            g_sb = [pp.tile([128, 1], F32, tag=f"g{l}", name=f"g{l}_sb") for l in range(3)]
            be_sb = [pp.tile([128, 1], F32, tag=f"be{l}", name=f"be{l}_sb") for l in range(3)]

            nc.sync.dma_start(h_sb[:], x_t.ap())
            nc.sync.dma_start(ld_sb[:], ld_d.ap())
            for h in (0, 1):
                nc.sync.dma_start(idx_sb[h][:], idx_d[h].ap())
            nc.sync.dma_start(dinv_col[:], dinv_col_d.ap())
            nc.sync.dma_start(dinv_b[:], dinv_b_d.ap())
            nc.sync.dma_start(iota_sb[:], iota_d.ap())
            nc.sync.dma_start(eye2_sb[:], eye2_d.ap())
            nc.sync.dma_start(id_sb[:], id_d.ap())
            for l in range(3):
                nc.sync.dma_start(w_sb[l][:], w_d[l].ap())
                nc.sync.dma_start(g_sb[l][:], g_d[l].ap())
                nc.sync.dma_start(be_sb[l][:], be_d[l].ap())

            u_loc_ap = u_loc.ap().rearrange("(g p) f -> p g f", p=128)

            for l in range(3):
                # ---- u_rows = dinv[n] * (h @ W) in [node, feat] layout ----
                for g in range(c.NB):
                    pu = pup.tile([128, 128], F32, tag="pu")
                    nc.tensor.matmul(pu[:], lhsT=h_sb[:, g * 128:(g + 1) * 128],
                                     rhs=w_sb[l][:], start=True, stop=True)
                    nc.vector.tensor_scalar(u_rows[:, g * 128:(g + 1) * 128], pu[:],
                                            dinv_col[:, g:g + 1], None,
                                            mybir.AluOpType.mult)
                nc.sync.dma_start(u_loc_ap, u_rows[:].rearrange("p (g f) -> p g f", f=128))

                if "ag" in stages:
                    nc.gpsimd.collective_compute(
                        "AllGather", mybir.AluOpType.bypass, replica_groups=rg,
                        ins=[u_loc.ap()[0:c.NPC, :]], outs=[u_full.ap()],
                    )

                # ---- gather message chunks ----
                msg_tiles = [[], []]
                qn = 0
                for h in (0, 1) if "gather" in stages else ():
                    src_ap = u_full.ap()[0:c.HALF, :] if h == 0 else u_full.ap()[c.HALF:c.N, :]
                    pool = m0p if h == 0 else m1p
                    for (t0, ctn) in chunks[h]:
                        mt = pool.tile([128, c.CT, 128], BF16, tag=f"m{h}", name=f"m{h}_chunk")
                        nidx = ctn * 128
                        nc.gpsimd.dma_gather(
                            out_ap=mt[:, 0:ctn, :],
                            in_ap=src_ap,
                            idxs_ap=idx_sb[h][:, t0 * 8:(t0 + ctn) * 8],
                            num_idxs=nidx,
                            num_idxs_reg=nidx,
                            elem_size=128,
                            queue_num=qn % 4,
                        )
                        qn += 1
                        msg_tiles[h].append((t0, ctn, mt))

                def msg_slice(h, gtile):
                    for (t0, ctn, mt) in msg_tiles[h]:
                        if t0 <= gtile < t0 + ctn:
                            return mt[:, gtile - t0, :]
                    raise AssertionError

                # ---- windows: one-hot + matmul accumulate ----
                NTMAX = int(nt.max())
                tcol = 0
                for w in range(c.W if "agg" in stages else 0):
                    pw = pwp.tile([128, 128], F32, tag="pw")
                    first = True
                    for h in (0, 1):
                        ntg = int(nt[w, h])
                        s_g = sp.tile([128, NTMAX, 128], BF16, tag="s", name="s_g")
                        iota_bc = iota_sb[:].rearrange("p (a b) -> p a b", a=1) \
                            .to_broadcast([128, ntg, 128])
                        ld_bc = ld_sb[:, tcol:tcol + ntg] \
                            .rearrange("p (a b) -> p a b", b=1) \
                            .to_broadcast([128, ntg, 128])
                        nc.vector.tensor_tensor(s_g[:, 0:ntg, :], iota_bc, ld_bc,
                                                mybir.AluOpType.is_equal)
                        for k in range(ntg):
                            gtile = int(meta["off_tiles"][w, h]) + k
                            nc.tensor.matmul(pw[:], lhsT=msg_slice(h, gtile),
                                             rhs=s_g[:, k, :],
                                             start=first, stop=False)
                            first = False
                            tcol += 1
                    nc.tensor.matmul(pw[:], lhsT=u_rows[:, w * 128:(w + 1) * 128],
                                     rhs=eye2_sb[:], start=first, stop=True)
                    wd = min(128, c.NPC - w * 128)
                    nc.vector.tensor_mul(agg[:, w * 128:w * 128 + wd], pw[:, 0:wd],
                                         dinv_b[:, w * 128:w * 128 + wd])

                if "agg" not in stages:
                    nc.vector.memset(agg[:], 0.01 * (l + 1))
                if "bn" not in stages:
                    if "bn1" in stages:
                        sum_c = cp.tile([128, 1], F32, tag="sum")
                        ssq_c = cp.tile([128, 1], F32, tag="ssq")
                        nc.vector.tensor_reduce(sum_c[:], agg[:, 0:c.NPC],
                                                axis=mybir.AxisListType.X, op=mybir.AluOpType.add)
                        if "ttr" in stages:
                            nc.vector.tensor_tensor_reduce(
                                h_sb[:, 0:c.NPC], agg[:, 0:c.NPC], agg[:, 0:c.NPC], 1.0, 0.0,
                                mybir.AluOpType.mult, mybir.AluOpType.add, ssq_c[:])
                        else:
                            nc.vector.tensor_reduce(ssq_c[:], agg[:, 0:c.NPC],
                                                    axis=mybir.AxisListType.X, op=mybir.AluOpType.add)
                        st_sb = cp.tile([128, 2], F32, tag="st")
                        nc.vector.tensor_copy(st_sb[:, 0:1], sum_c[:])
                        nc.vector.tensor_copy(st_sb[:, 1:2], ssq_c[:])
                        nc.sync.dma_start(stats_in.ap(), st_sb[:])
                        nc.gpsimd.collective_compute(
                            "AllReduce", mybir.AluOpType.add, replica_groups=rg,
                            ins=[stats_in.ap()], outs=[stats_out.ap()],
                        )
                        st2 = cp.tile([128, 2], F32, tag="st2")
                        nc.sync.dma_start(st2[:], stats_out.ap())
                    nc.scalar.activation(h_sb[:, 0:c.NPC], agg[:, 0:c.NPC],
                                         mybir.ActivationFunctionType.Relu)
                    continue
                # ---- BatchNorm (training stats over all N nodes) + ReLU ----
                sum_c = cp.tile([128, 1], F32, tag="sum")
                ssq_c = cp.tile([128, 1], F32, tag="ssq")
                nc.vector.tensor_reduce(sum_c[:], agg[:, 0:c.NPC],
                                        axis=mybir.AxisListType.X, op=mybir.AluOpType.add)
                nc.vector.tensor_mul(h_sb[:, 0:c.NPC], agg[:, 0:c.NPC], agg[:, 0:c.NPC])
                nc.vector.tensor_reduce(ssq_c[:], h_sb[:, 0:c.NPC],
                                        axis=mybir.AxisListType.X, op=mybir.AluOpType.add)
                st_sb = cp.tile([128, 2], F32, tag="st")
                nc.vector.tensor_copy(st_sb[:, 0:1], sum_c[:])
                nc.vector.tensor_copy(st_sb[:, 1:2], ssq_c[:])
                nc.sync.dma_start(stats_in.ap(), st_sb[:])
                nc.gpsimd.collective_compute(
                    "AllReduce", mybir.AluOpType.add, replica_groups=rg,
                    ins=[stats_in.ap()], outs=[stats_out.ap()],
                )
                st2 = cp.tile([128, 2], F32, tag="st2")
                nc.sync.dma_start(st2[:], stats_out.ap())
                mean_c = cp.tile([128, 1], F32, tag="mean")
                var_c = cp.tile([128, 1], F32, tag="var")
                t1 = cp.tile([128, 1], F32, tag="t1")
                s_col = cp.tile([128, 1], F32, tag="scol")
                t_col = cp.tile([128, 1], F32, tag="tcol")
                inv_n = 1.0 / float(c.N)
                nc.vector.tensor_scalar_mul(mean_c[:], st2[:, 0:1], inv_n)
                nc.vector.tensor_scalar_mul(var_c[:], st2[:, 1:2], inv_n)   # E[x^2]
                nc.vector.tensor_mul(t1[:], mean_c[:], mean_c[:])
                nc.vector.tensor_sub(var_c[:], var_c[:], t1[:])
                nc.vector.tensor_scalar_add(var_c[:], var_c[:], BN_EPS)
                nc.scalar.sqrt(t1[:], var_c[:])
                nc.vector.reciprocal(t1[:], t1[:])                           # rstd
                nc.vector.tensor_mul(s_col[:], g_sb[l][:], t1[:])
                nc.vector.tensor_mul(t_col[:], mean_c[:], s_col[:])
                nc.vector.tensor_sub(t_col[:], be_sb[l][:], t_col[:])
                # h = relu(agg * s + t)
                nc.scalar.activation(h_sb[:, 0:c.NPC], agg[:, 0:c.NPC],
                                     mybir.ActivationFunctionType.Relu,
                                     bias=t_col[:], scale=s_col[:])

            # ---- output: transpose h -> [node, feat] and store ----
            for g in range(c.NB if "out" in stages else 0):
                pt = pup.tile([128, 128], F32, tag="pu")
                nc.tensor.transpose(pt[:], h_sb[:, g * 128:(g + 1) * 128], id_sb[:])
                nc.vector.tensor_copy(agg[:, g * 128:(g + 1) * 128], pt[:])
            nc.sync.dma_start(y_d.ap().rearrange("(g p) f -> p g f", p=128),
                              agg[:].rearrange("p (g f) -> p g f", f=128))

    nc.compile()
    return nc


_CACHE = {}


def _get_program(cfg, meta):
    key = (cfg.N, cfg.E, cfg.NC, cfg.CT, meta["NT"],
           tuple(meta["s_tiles"].tolist()), meta["nt"].tobytes())
    if key not in _CACHE:
        _CACHE[key] = build(cfg, meta)
    return _CACHE[key]


def run(cfg, inputs, trace=False):
    Ws = [inputs[f"W{l}"] for l in range(3)]
    gs = [inputs[f"g{l}"] for l in range(3)]
    bes = [inputs[f"be{l}"] for l in range(3)]
    in_maps, meta = preprocess(cfg, inputs["x"], inputs["edge_index"], Ws, gs, bes)
    nc = _get_program(cfg, meta)
    res = bass_utils.run_bass_kernel_spmd(
        nc, in_maps, core_ids=list(range(cfg.NC)), trace=trace)
    out = np.concatenate([res.results[ci]["y"][:cfg.NPC] for ci in range(cfg.NC)], axis=0)
    return out.astype(np.float32), res


def kernel(**inputs):
    cfg = Cfg()
    out, _ = run(cfg, inputs)
    return out


# revision 10
# speedup vs baseline: 2.6418x; 1.4083x over previous
"""GCN 3-layer (improved GCNConv + BN + ReLU) Bass/Tile kernel for 8 trn2 NeuronCores.

Strategy (graph/data parallel, per sharding hint):
  - Nodes partitioned into 8 contiguous ranges (6250/core); edges routed to the
    core owning their destination node (host-side routing = the sharding step).
  - Per layer: each core computes u = dinv[n] * (h @ W) for its nodes (PE),
    AllGathers u (bf16, [N,128]) into HBM, then per 128-edge tile:
      * dma_gather pulls u[src] rows into SBUF (messages, bf16)
      * DVE builds a one-hot matrix S[e, dst_slot] = (localdst[e] == iota) (bf16)
      * PE accumulates psum[feat, dst_slot] += msg.T @ S over the ~35 tiles of a
        128-destination window; the self-loop term is folded in as one extra
        matmul with lhsT=u_rows[window], rhs=2*I.
    The int16 gather-index limit is handled by splitting the u table at row
    32768 into two gather sources ("halves").
  - agg *= dinv[dst] (broadcast tile), then BN stats (free-dim reduce +
    AllReduce [128,2]) and fused scale/shift+ReLU on ScalarE.
  - h lives as [128 feat, nodes] f32 in SBUF the whole time; output is
    PE-transposed back to [nodes, 128].

All per-core program structure (tile counts per window/half) is padded to the
max across cores so one SPMD program serves all 8 cores; pads use gather row 0
and localdst=200 (one-hot row of zeros -> no contribution).
"""

import numpy as np
import ml_dtypes

import concourse.bass as bass
import concourse.bacc as bacc
import concourse.tile as tile
from concourse import bass_utils, mybir

F32 = mybir.dt.float32
BF16 = mybir.dt.bfloat16
I16 = mybir.dt.int16
BF16_NP = ml_dtypes.bfloat16

BN_EPS = 1e-5
PAD_LD = 200.0  # localdst value for pad slots; > 127 so one-hot row is all zero


class Cfg:
    def __init__(self, n_nodes=50000, n_edges=1600000, n_cores=8, chunk_tiles=8,
                 half=32768):
        self.N = n_nodes
        self.E = n_edges
        self.NC = n_cores
        self.D = 128
        assert n_nodes % n_cores == 0
        self.NPC = n_nodes // n_cores          # nodes per core
        self.W = (self.NPC + 127) // 128       # windows (= node blocks) per core
        self.NB = self.W                       # alias
        self.NPAD = self.W * 128               # padded nodes per core
        self.HALF = half                       # u-table split row (int16 idx limit)
        self.CT = chunk_tiles                  # gather chunk size in 128-edge tiles


def preprocess(cfg, x, edge_index, Ws, gs, bes):
    """Host-side edge routing/sharding. Returns (in_maps, meta)."""
    c = cfg
    src = np.asarray(edge_index[0]).astype(np.int64)
    dst = np.asarray(edge_index[1]).astype(np.int64)
    x = np.asarray(x, dtype=np.float32)

    deg = (np.bincount(dst, minlength=c.N) + 2.0).astype(np.float32)
    dinv = (1.0 / np.sqrt(deg)).astype(np.float32)

    core = dst // c.NPC
    wloc = (dst % c.NPC) // 128
    half = (src >= c.HALF).astype(np.int64)

    gkey = (core * c.W + wloc) * 2 + half
    cnt = np.bincount(gkey, minlength=c.NC * c.W * 2).reshape(c.NC, c.W, 2)
    # tiles per (window, half) group -- shared across cores (max + ceil)
    nt = np.ceil(cnt.max(axis=0) / 128.0).astype(np.int64)  # [W, 2]
    nt = np.maximum(nt, 1)
    s_tiles = nt.sum(axis=0)  # [2] tiles per half-stream
    off_tiles = np.cumsum(nt, axis=0) - nt  # [W, 2] tile offset within half-stream
    ntw = nt[:, 0] + nt[:, 1]
    ldcol0 = np.cumsum(ntw) - ntw           # [W] first localdst column of window
    NT = int(ntw.sum())

    meta = dict(nt=nt, s_tiles=s_tiles, off_tiles=off_tiles, ldcol0=ldcol0, NT=NT)

    # constants (identical for every core)
    iota_b = np.tile(np.arange(128, dtype=np.float32)[None, :], (128, 1)).astype(BF16_NP)
    eye2_b = (2.0 * np.eye(128, dtype=np.float32)).astype(BF16_NP)
    id_f = np.eye(128, dtype=np.float32)

    in_maps = []
    for ci in range(c.NC):
        base = ci * c.NPC
        m = core == ci
        s_c = src[m]
        d_c = dst[m]
        wv = wloc[m]
        hv = half[m]
        gk = wv * 2 + hv
        order = np.argsort(gk, kind="stable")
        s_c, d_c, wv, hv, gk = s_c[order], d_c[order], wv[order], hv[order], gk[order]
        cnt_c = np.bincount(gk, minlength=c.W * 2)
        gstart = np.cumsum(cnt_c) - cnt_c
        rank = np.arange(s_c.size) - gstart[gk]

        idx_flat = [np.zeros(int(s_tiles[0]) * 128, np.int16),
                    np.zeros(int(s_tiles[1]) * 128, np.int16)]
        pos_slot = off_tiles[wv, hv] * 128 + rank
        for h in (0, 1):
            mh = hv == h
            idx_flat[h][pos_slot[mh]] = (s_c[mh] - h * c.HALF).astype(np.int16)

        ld_flat = np.full(meta["NT"] * 128, PAD_LD, np.float32)
        ldpos = (ldcol0[wv] + hv * nt[wv, 0] + rank // 128) * 128 + rank % 128
        ld_flat[ldpos] = (d_c - (base + wv * 128)).astype(np.float32)

        def wrap16(a):
            t = a.reshape(-1, 16).T  # [16, S/16], element (p, j) = a[16j+p]
            return np.ascontiguousarray(np.tile(t, (8, 1)))

        dv = np.zeros(c.NPAD, np.float32)
        dv[:c.NPC] = dinv[base:base + c.NPC]
        xt = np.zeros((128, c.NPAD), np.float32)
        xt[:, :c.NPC] = x[base:base + c.NPC].T

        im = {
            "x_t": xt,
            "idx0": wrap16(idx_flat[0]),
            "idx1": wrap16(idx_flat[1]),
            "ld": np.ascontiguousarray(ld_flat.reshape(meta["NT"], 128).T).astype(BF16_NP),
            "dinv_col": np.ascontiguousarray(dv.reshape(c.NB, 128).T),
            "dinv_b": np.ascontiguousarray(np.tile(dv[None, :], (128, 1))),
            "iota_b": iota_b,
            "eye2_b": eye2_b,
            "id_f": id_f,
        }
        for li in range(3):
            im[f"w{li}"] = np.asarray(Ws[li], np.float32)
            im[f"g{li}"] = np.asarray(gs[li], np.float32).reshape(128, 1)
            im[f"be{li}"] = np.asarray(bes[li], np.float32).reshape(128, 1)
        in_maps.append(im)
    return in_maps, meta


def build(cfg, meta, stages=("u", "ag", "gather", "agg", "bn", "out")):
    stages = set(stages)
    c = cfg
    nt = meta["nt"]
    s_tiles = meta["s_tiles"]
    NT = meta["NT"]

    nc = bacc.Bacc("TRN2", target_bir_lowering=False, debug=False, num_devices=c.NC,
                   num_swdge_queues=4)

    x_t = nc.dram_tensor("x_t", [128, c.NPAD], F32, kind="ExternalInput")
    idx_d = [nc.dram_tensor(f"idx{h}", [128, int(s_tiles[h]) * 8], I16, kind="ExternalInput")
             for h in (0, 1)]
    ld_d = nc.dram_tensor("ld", [128, NT], BF16, kind="ExternalInput")
    dinv_col_d = nc.dram_tensor("dinv_col", [128, c.NB], F32, kind="ExternalInput")
    dinv_b_d = nc.dram_tensor("dinv_b", [128, c.NPAD], F32, kind="ExternalInput")
    iota_d = nc.dram_tensor("iota_b", [128, 128], BF16, kind="ExternalInput")
    eye2_d = nc.dram_tensor("eye2_b", [128, 128], BF16, kind="ExternalInput")
    id_d = nc.dram_tensor("id_f", [128, 128], F32, kind="ExternalInput")
    w_d = [nc.dram_tensor(f"w{l}", [128, 128], F32, kind="ExternalInput") for l in range(3)]
    g_d = [nc.dram_tensor(f"g{l}", [128, 1], F32, kind="ExternalInput") for l in range(3)]
    be_d = [nc.dram_tensor(f"be{l}", [128, 1], F32, kind="ExternalInput") for l in range(3)]

    y_d = nc.dram_tensor("y", [c.NPAD, 128], F32, kind="ExternalOutput")

    shared = "Shared" if c.NC > 4 else "Local"
    u_loc = nc.dram_tensor("u_loc", [c.NPAD, 128], BF16, kind="Internal")
    u_full = nc.dram_tensor("u_full", [c.N, 128], BF16, kind="Internal", addr_space=shared)
    stats_in = nc.dram_tensor("stats_in", [128, 2], F32, kind="Internal")
    stats_out = nc.dram_tensor("stats_out", [128, 2], F32, kind="Internal", addr_space=shared)

    rg = [list(range(c.NC))]

    # gather chunk layout per half: list of (tile_start, n_tiles)
    chunks = []
    for h in (0, 1):
        S = int(s_tiles[h])
        ch = []
        t0 = 0
        while t0 < S:
            ch.append((t0, min(c.CT, S - t0)))
            t0 += min(c.CT, S - t0)
        chunks.append(ch)

    with tile.TileContext(nc) as tc:
        with (
            tc.tile_pool(name="persist", bufs=1) as pp,
            tc.tile_pool(name="msg0", bufs=8) as m0p,
            tc.tile_pool(name="msg1", bufs=8) as m1p,
            tc.tile_pool(name="onehot", bufs=3) as sp,
            tc.tile_pool(name="cols", bufs=1) as cp,
            tc.tile_pool(name="pw", bufs=4, space="PSUM") as pwp,
            tc.tile_pool(name="pu", bufs=2, space="PSUM") as pup,
        ):
            # ---- persistent SBUF ----
            h_sb = pp.tile([128, c.NPAD], F32, tag="h")
            u_rows = pp.tile([128, c.NPAD], BF16, tag="u_rows")
            agg = pp.tile([128, c.NPAD], F32, tag="agg")
            dinv_b = pp.tile([128, c.NPAD], F32, tag="dinv_b")
            ld_sb = pp.tile([128, NT], BF16, tag="ld")
            idx_sb = [pp.tile([128, int(s_tiles[h]) * 8], I16, tag=f"idx{h}", name=f"idx{h}_sb") for h in (0, 1)]
            dinv_col = pp.tile([128, c.NB], F32, tag="dinv_col")
            iota_sb = pp.tile([128, 128], BF16, tag="iota")
            eye2_sb = pp.tile([128, 128], BF16, tag="eye2")
            id_sb = pp.tile([128, 128], F32, tag="idf")
            w_sb = [pp.tile([128, 128], F32, tag=f"w{l}", name=f"w{l}_sb") for l in range(3)]
            g_sb = [pp.tile([128, 1], F32, tag=f"g{l}", name=f"g{l}_sb") for l in range(3)]
            be_sb = [pp.tile([128, 1], F32, tag=f"be{l}", name=f"be{l}_sb") for l in range(3)]

            nc.sync.dma_start(h_sb[:], x_t.ap())
            nc.sync.dma_start(ld_sb[:], ld_d.ap())
            for h in (0, 1):
                nc.sync.dma_start(idx_sb[h][:], idx_d[h].ap())
            nc.sync.dma_start(dinv_col[:], dinv_col_d.ap())
            nc.sync.dma_start(dinv_b[:], dinv_b_d.ap())
            nc.sync.dma_start(iota_sb[:], iota_d.ap())
            nc.sync.dma_start(eye2_sb[:], eye2_d.ap())
            nc.sync.dma_start(id_sb[:], id_d.ap())
            for l in range(3):
                nc.sync.dma_start(w_sb[l][:], w_d[l].ap())
                nc.sync.dma_start(g_sb[l][:], g_d[l].ap())
                nc.sync.dma_start(be_sb[l][:], be_d[l].ap())

            u_loc_ap = u_loc.ap().rearrange("(g p) f -> p g f", p=128)

            for l in range(3):
                # ---- u_rows = dinv[n] * (h @ W) in [node, feat] layout ----
                for g in range(c.NB):
                    pu = pup.tile([128, 128], F32, tag="pu")
                    nc.tensor.matmul(pu[:], lhsT=h_sb[:, g * 128:(g + 1) * 128],
                                     rhs=w_sb[l][:], start=True, stop=True)
                    nc.vector.tensor_scalar(u_rows[:, g * 128:(g + 1) * 128], pu[:],
                                            dinv_col[:, g:g + 1], None,
                                            mybir.AluOpType.mult)
                nc.sync.dma_start(u_loc_ap, u_rows[:].rearrange("p (g f) -> p g f", f=128))

                if "ag" in stages:
                    nc.gpsimd.collective_compute(
                        "AllGather", mybir.AluOpType.bypass, replica_groups=rg,
                        ins=[u_loc.ap()[0:c.NPC, :]], outs=[u_full.ap()],
                    )

                # ---- gather message chunks ----
                msg_tiles = [[], []]
                qn = 0
                for h in (0, 1) if "gather" in stages else ():
                    src_ap = u_full.ap()[0:c.HALF, :] if h == 0 else u_full.ap()[c.HALF:c.N, :]
                    pool = m0p if h == 0 else m1p
                    for (t0, ctn) in chunks[h]:
                        mt = pool.tile([128, c.CT, 128], BF16, tag=f"m{h}", name=f"m{h}_chunk")
                        nidx = ctn * 128
                        nc.gpsimd.dma_gather(
                            out_ap=mt[:, 0:ctn, :],
                            in_ap=src_ap,
                            idxs_ap=idx_sb[h][:, t0 * 8:(t0 + ctn) * 8],
                            num_idxs=nidx,
                            num_idxs_reg=nidx,
                            elem_size=128,
                            queue_num=qn % 4,
                        )
                        qn += 1
                        msg_tiles[h].append((t0, ctn, mt))

                def msg_slice(h, gtile):
                    for (t0, ctn, mt) in msg_tiles[h]:
                        if t0 <= gtile < t0 + ctn:
                            return mt[:, gtile - t0, :]
                    raise AssertionError

                # ---- windows: one-hot + matmul accumulate ----
                NTMAX = int(nt.max())
                tcol = 0
                for w in range(c.W if "agg" in stages else 0):
                    pw = pwp.tile([128, 128], F32, tag="pw")
                    first = True
                    for h in (0, 1):
                        ntg = int(nt[w, h])
                        s_g = sp.tile([128, NTMAX, 128], BF16, tag="s", name="s_g")
                        iota_bc = iota_sb[:].rearrange("p (a b) -> p a b", a=1) \
                            .to_broadcast([128, ntg, 128])
                        ld_bc = ld_sb[:, tcol:tcol + ntg] \
                            .rearrange("p (a b) -> p a b", b=1) \
                            .to_broadcast([128, ntg, 128])
                        nc.vector.tensor_tensor(s_g[:, 0:ntg, :], iota_bc, ld_bc,
                                                mybir.AluOpType.is_equal)
                        for k in range(ntg):
                            gtile = int(meta["off_tiles"][w, h]) + k
                            nc.tensor.matmul(pw[:], lhsT=msg_slice(h, gtile),
                                             rhs=s_g[:, k, :],
                                             start=first, stop=False)
                            first = False
                            tcol += 1
                    nc.tensor.matmul(pw[:], lhsT=u_rows[:, w * 128:(w + 1) * 128],
                                     rhs=eye2_sb[:], start=first, stop=True)
                    wd = min(128, c.NPC - w * 128)
                    nc.vector.tensor_mul(agg[:, w * 128:w * 128 + wd], pw[:, 0:wd],
                                         dinv_b[:, w * 128:w * 128 + wd])

                if "agg" not in stages:
                    nc.vector.memset(agg[:], 0.01 * (l + 1))
                if "bn" not in stages:
                    if "bn1" in stages:
                        sum_c = cp.tile([128, 1], F32, tag="sum")
                        ssq_c = cp.tile([128, 1], F32, tag="ssq")
                        nc.vector.tensor_reduce(sum_c[:], agg[:, 0:c.NPC],
                                                axis=mybir.AxisListType.X, op=mybir.AluOpType.add)
                        if "ttr" in stages:
                            nc.vector.tensor_tensor_reduce(
                                h_sb[:, 0:c.NPC], agg[:, 0:c.NPC], agg[:, 0:c.NPC], 1.0, 0.0,
                                mybir.AluOpType.mult, mybir.AluOpType.add, ssq_c[:])
                        else:
                            nc.vector.tensor_reduce(ssq_c[:], agg[:, 0:c.NPC],
                                                    axis=mybir.AxisListType.X, op=mybir.AluOpType.add)
                        st_sb = cp.tile([128, 2], F32, tag="st")
                        nc.vector.tensor_copy(st_sb[:, 0:1], sum_c[:])
                        nc.vector.tensor_copy(st_sb[:, 1:2], ssq_c[:])
                        nc.sync.dma_start(stats_in.ap(), st_sb[:])
                        nc.gpsimd.collective_compute(
                            "AllReduce", mybir.AluOpType.add, replica_groups=rg,
                            ins=[stats_in.ap()], outs=[stats_out.ap()],
                        )
                        st2 = cp.tile([128, 2], F32, tag="st2")
                        nc.sync.dma_start(st2[:], stats_out.ap())
                    nc.scalar.activation(h_sb[:, 0:c.NPC], agg[:, 0:c.NPC],
                                         mybir.ActivationFunctionType.Relu)
                    continue
                # ---- BatchNorm (training stats over all N nodes) + ReLU ----
                sum_c = cp.tile([128, 1], F32, tag="sum")
                ssq_c = cp.tile([128, 1], F32, tag="ssq")
                nc.vector.tensor_reduce(sum_c[:], agg[:, 0:c.NPC],
                                        axis=mybir.AxisListType.X, op=mybir.AluOpType.add)
                nc.vector.tensor_mul(h_sb[:, 0:c.NPC], agg[:, 0:c.NPC], agg[:, 0:c.NPC])
                nc.vector.tensor_reduce(ssq_c[:], h_sb[:, 0:c.NPC],
                                        axis=mybir.AxisListType.X, op=mybir.AluOpType.add)
                st_sb = cp.tile([128, 2], F32, tag="st")
                nc.vector.tensor_copy(st_sb[:, 0:1], sum_c[:])
                nc.vector.tensor_copy(st_sb[:, 1:2], ssq_c[:])
                nc.sync.dma_start(stats_in.ap(), st_sb[:])
                nc.gpsimd.collective_compute(
                    "AllReduce", mybir.AluOpType.add, replica_groups=rg,
                    ins=[stats_in.ap()], outs=[stats_out.ap()],
                )
                st2 = cp.tile([128, 2], F32, tag="st2")
                nc.sync.dma_start(st2[:], stats_out.ap())
                mean_c = cp.tile([128, 1], F32, tag="mean")
                var_c = cp.tile([128, 1], F32, tag="var")
                t1 = cp.tile([128, 1], F32, tag="t1")
                s_col = cp.tile([128, 1], F32, tag="scol")
                t_col = cp.tile([128, 1], F32, tag="tcol")
                inv_n = 1.0 / float(c.N)
                nc.vector.tensor_scalar_mul(mean_c[:], st2[:, 0:1], inv_n)
                nc.vector.tensor_scalar_mul(var_c[:], st2[:, 1:2], inv_n)   # E[x^2]
                nc.vector.tensor_mul(t1[:], mean_c[:], mean_c[:])
                nc.vector.tensor_sub(var_c[:], var_c[:], t1[:])
                nc.vector.tensor_scalar_add(var_c[:], var_c[:], BN_EPS)
                nc.scalar.sqrt(t1[:], var_c[:])
                nc.vector.reciprocal(t1[:], t1[:])                           # rstd
                nc.vector.tensor_mul(s_col[:], g_sb[l][:], t1[:])
                nc.vector.tensor_mul(t_col[:], mean_c[:], s_col[:])
                nc.vector.tensor_sub(t_col[:], be_sb[l][:], t_col[:])
                # h = relu(agg * s + t)
                nc.scalar.activation(h_sb[:, 0:c.NPC], agg[:, 0:c.NPC],
                                     mybir.ActivationFunctionType.Relu,
                                     bias=t_col[:], scale=s_col[:])

            # ---- output: transpose h -> [node, feat] and store ----
            for g in range(c.NB if "out" in stages else 0):
                pt = pup.tile([128, 128], F32, tag="pu")
                nc.tensor.transpose(pt[:], h_sb[:, g * 128:(g + 1) * 128], id_sb[:])
                nc.vector.tensor_copy(agg[:, g * 128:(g + 1) * 128], pt[:])
            nc.sync.dma_start(y_d.ap().rearrange("(g p) f -> p g f", p=128),
                              agg[:].rearrange("p (g f) -> p g f", f=128))

    nc.compile()
    return nc


_CACHE = {}


def _get_program(cfg, meta):
    key = (cfg.N, cfg.E, cfg.NC, cfg.CT, meta["NT"],
           tuple(meta["s_tiles"].tolist()), meta["nt"].tobytes())
    if key not in _CACHE:
        _CACHE[key] = build(cfg, meta)
    return _CACHE[key]


def run(cfg, inputs, trace=False):
    Ws = [inputs[f"W{l}"] for l in range(3)]
    gs = [inputs[f"g{l}"] for l in range(3)]
    bes = [inputs[f"be{l}"] for l in range(3)]
    in_maps, meta = preprocess(cfg, inputs["x"], inputs["edge_index"], Ws, gs, bes)
    nc = _get_program(cfg, meta)
    res = bass_utils.run_bass_kernel_spmd(
        nc, in_maps, core_ids=list(range(cfg.NC)), trace=trace)
    out = np.concatenate([res.results[ci]["y"][:cfg.NPC] for ci in range(cfg.NC)], axis=0)
    return out.astype(np.float32), res


def kernel(**inputs):
    cfg = Cfg()
    out, _ = run(cfg, inputs)
    return out


# revision 11
# speedup vs baseline: 2.6927x; 1.0193x over previous
"""GCN 3-layer (improved GCNConv + BN + ReLU) Bass/Tile kernel for 8 trn2 NeuronCores.

Strategy (graph/data parallel, per sharding hint):
  - Nodes partitioned into 8 contiguous ranges (6250/core); edges routed to the
    core owning their destination node (host-side routing = the sharding step).
  - Per layer: each core computes u = dinv[n] * (h @ W) for its nodes (PE),
    AllGathers u (bf16, [N,128]) into HBM, then per 128-edge tile:
      * dma_gather pulls u[src] rows into SBUF (messages, bf16)
      * DVE builds a one-hot matrix S[e, dst_slot] = (localdst[e] == iota) (bf16)
      * PE accumulates psum[feat, dst_slot] += msg.T @ S over the ~35 tiles of a
        128-destination window; the self-loop term is folded in as one extra
        matmul with lhsT=u_rows[window], rhs=2*I.
    The int16 gather-index limit is handled by splitting the u table at row
    32768 into two gather sources ("halves").
  - agg *= dinv[dst] (broadcast tile), then BN stats (free-dim reduce +
    AllReduce [128,2]) and fused scale/shift+ReLU on ScalarE.
  - h lives as [128 feat, nodes] f32 in SBUF the whole time; output is
    PE-transposed back to [nodes, 128].

All per-core program structure (tile counts per window/half) is padded to the
max across cores so one SPMD program serves all 8 cores; pads use gather row 0
and localdst=200 (one-hot row of zeros -> no contribution).
"""

import numpy as np
import ml_dtypes

import concourse.bass as bass
import concourse.bacc as bacc
import concourse.tile as tile
from concourse import bass_utils, mybir

F32 = mybir.dt.float32
BF16 = mybir.dt.bfloat16
I16 = mybir.dt.int16
BF16_NP = ml_dtypes.bfloat16

BN_EPS = 1e-5
PAD_LD = 200.0  # localdst value for pad slots; > 127 so one-hot row is all zero


class Cfg:
    def __init__(self, n_nodes=50000, n_edges=1600000, n_cores=8, chunk_tiles=8,
                 half=32768):
        self.N = n_nodes
        self.E = n_edges
        self.NC = n_cores
        self.D = 128
        assert n_nodes % n_cores == 0
        self.NPC = n_nodes // n_cores          # nodes per core
        self.W = (self.NPC + 127) // 128       # windows (= node blocks) per core
        self.NB = self.W                       # alias
        self.NPAD = self.W * 128               # padded nodes per core
        self.HALF = half                       # u-table split row (int16 idx limit)
        self.CT = chunk_tiles                  # gather chunk size in 128-edge tiles


def preprocess(cfg, x, edge_index, Ws, gs, bes):
    """Host-side edge routing/sharding. Returns (in_maps, meta)."""
    c = cfg
    src = np.asarray(edge_index[0]).astype(np.int64)
    dst = np.asarray(edge_index[1]).astype(np.int64)
    x = np.asarray(x, dtype=np.float32)

    deg = (np.bincount(dst, minlength=c.N) + 2.0).astype(np.float32)
    dinv = (1.0 / np.sqrt(deg)).astype(np.float32)

    core = dst // c.NPC
    wloc = (dst % c.NPC) // 128
    half = (src >= c.HALF).astype(np.int64)

    gkey = (core * c.W + wloc) * 2 + half
    cnt = np.bincount(gkey, minlength=c.NC * c.W * 2).reshape(c.NC, c.W, 2)
    # tiles per (window, half) group -- shared across cores (max + ceil)
    nt = np.ceil(cnt.max(axis=0) / 128.0).astype(np.int64)  # [W, 2]
    nt = np.maximum(nt, 1)
    s_tiles = nt.sum(axis=0)  # [2] tiles per half-stream
    off_tiles = np.cumsum(nt, axis=0) - nt  # [W, 2] tile offset within half-stream
    ntw = nt[:, 0] + nt[:, 1]
    ldcol0 = np.cumsum(ntw) - ntw           # [W] first localdst column of window
    NT = int(ntw.sum())

    meta = dict(nt=nt, s_tiles=s_tiles, off_tiles=off_tiles, ldcol0=ldcol0, NT=NT)

    # constants (identical for every core)
    iota_b = np.tile(np.arange(128, dtype=np.float32)[None, :], (128, 1)).astype(BF16_NP)
    eye2_b = (2.0 * np.eye(128, dtype=np.float32)).astype(BF16_NP)
    id_f = np.eye(128, dtype=np.float32)

    in_maps = []
    for ci in range(c.NC):
        base = ci * c.NPC
        m = core == ci
        s_c = src[m]
        d_c = dst[m]
        wv = wloc[m]
        hv = half[m]
        gk = wv * 2 + hv
        order = np.argsort(gk, kind="stable")
        s_c, d_c, wv, hv, gk = s_c[order], d_c[order], wv[order], hv[order], gk[order]
        cnt_c = np.bincount(gk, minlength=c.W * 2)
        gstart = np.cumsum(cnt_c) - cnt_c
        rank = np.arange(s_c.size) - gstart[gk]

        idx_flat = [np.zeros(int(s_tiles[0]) * 128, np.int16),
                    np.zeros(int(s_tiles[1]) * 128, np.int16)]
        pos_slot = off_tiles[wv, hv] * 128 + rank
        for h in (0, 1):
            mh = hv == h
            idx_flat[h][pos_slot[mh]] = (s_c[mh] - h * c.HALF).astype(np.int16)

        ld_flat = np.full(meta["NT"] * 128, PAD_LD, np.float32)
        ldpos = (ldcol0[wv] + hv * nt[wv, 0] + rank // 128) * 128 + rank % 128
        ld_flat[ldpos] = (d_c - (base + wv * 128)).astype(np.float32)

        def wrap16(a):
            t = a.reshape(-1, 16).T  # [16, S/16], element (p, j) = a[16j+p]
            return np.ascontiguousarray(np.tile(t, (8, 1)))

        dv = np.zeros(c.NPAD, np.float32)
        dv[:c.NPC] = dinv[base:base + c.NPC]
        xt = np.zeros((128, c.NPAD), np.float32)
        xt[:, :c.NPC] = x[base:base + c.NPC].T

        im = {
            "x_t": xt,
            "idx0": wrap16(idx_flat[0]),
            "idx1": wrap16(idx_flat[1]),
            "ld": np.ascontiguousarray(ld_flat.reshape(meta["NT"], 128).T).astype(BF16_NP),
            "dinv_col": np.ascontiguousarray(dv.reshape(c.NB, 128).T),
            "dinv_b": np.ascontiguousarray(np.tile(dv[None, :], (128, 1))),
            "iota_b": iota_b,
            "eye2_b": eye2_b,
            "id_f": id_f,
        }
        for li in range(3):
            im[f"w{li}"] = np.asarray(Ws[li], np.float32)
            im[f"g{li}"] = np.asarray(gs[li], np.float32).reshape(128, 1)
            im[f"be{li}"] = np.asarray(bes[li], np.float32).reshape(128, 1)
        in_maps.append(im)
    return in_maps, meta


def build(cfg, meta, stages=("u", "ag", "gather", "agg", "bn", "out")):
    stages = set(stages)
    c = cfg
    nt = meta["nt"]
    s_tiles = meta["s_tiles"]
    NT = meta["NT"]

    nc = bacc.Bacc("TRN2", target_bir_lowering=False, debug=False, num_devices=c.NC,
                   num_swdge_queues=4)

    x_t = nc.dram_tensor("x_t", [128, c.NPAD], F32, kind="ExternalInput")
    idx_d = [nc.dram_tensor(f"idx{h}", [128, int(s_tiles[h]) * 8], I16, kind="ExternalInput")
             for h in (0, 1)]
    ld_d = nc.dram_tensor("ld", [128, NT], BF16, kind="ExternalInput")
    dinv_col_d = nc.dram_tensor("dinv_col", [128, c.NB], F32, kind="ExternalInput")
    dinv_b_d = nc.dram_tensor("dinv_b", [128, c.NPAD], F32, kind="ExternalInput")
    iota_d = nc.dram_tensor("iota_b", [128, 128], BF16, kind="ExternalInput")
    eye2_d = nc.dram_tensor("eye2_b", [128, 128], BF16, kind="ExternalInput")
    id_d = nc.dram_tensor("id_f", [128, 128], F32, kind="ExternalInput")
    w_d = [nc.dram_tensor(f"w{l}", [128, 128], F32, kind="ExternalInput") for l in range(3)]
    g_d = [nc.dram_tensor(f"g{l}", [128, 1], F32, kind="ExternalInput") for l in range(3)]
    be_d = [nc.dram_tensor(f"be{l}", [128, 1], F32, kind="ExternalInput") for l in range(3)]

    y_d = nc.dram_tensor("y", [c.NPAD, 128], F32, kind="ExternalOutput")

    shared = "Shared" if c.NC > 4 else "Local"
    u_loc = nc.dram_tensor("u_loc", [c.NPAD, 128], BF16, kind="Internal")
    u_full = nc.dram_tensor("u_full", [c.N, 128], BF16, kind="Internal", addr_space=shared)
    stats_in = nc.dram_tensor("stats_in", [128, 2], F32, kind="Internal")
    stats_out = nc.dram_tensor("stats_out", [128, 2], F32, kind="Internal", addr_space=shared)

    rg = [list(range(c.NC))]

    # gather chunk layout per half: list of (tile_start, n_tiles)
    chunks = []
    for h in (0, 1):
        S = int(s_tiles[h])
        ch = []
        t0 = 0
        while t0 < S:
            ch.append((t0, min(c.CT, S - t0)))
            t0 += min(c.CT, S - t0)
        chunks.append(ch)

    with tile.TileContext(nc) as tc:
        with (
            tc.tile_pool(name="persist", bufs=1) as pp,
            tc.tile_pool(name="msg0", bufs=10) as m0p,
            tc.tile_pool(name="msg1", bufs=6) as m1p,
            tc.tile_pool(name="onehot", bufs=3) as sp,
            tc.tile_pool(name="cols", bufs=1) as cp,
            tc.tile_pool(name="pw", bufs=4, space="PSUM") as pwp,
            tc.tile_pool(name="pu", bufs=2, space="PSUM") as pup,
        ):
            # ---- persistent SBUF ----
            h_sb = pp.tile([128, c.NPAD], F32, tag="h")
            u_rows = pp.tile([128, c.NPAD], BF16, tag="u_rows")
            agg = pp.tile([128, c.NPAD], F32, tag="agg")
            dinv_b = pp.tile([128, c.NPAD], F32, tag="dinv_b")
            ld_sb = pp.tile([128, NT], BF16, tag="ld")
            idx_sb = [pp.tile([128, int(s_tiles[h]) * 8], I16, tag=f"idx{h}", name=f"idx{h}_sb") for h in (0, 1)]
            dinv_col = pp.tile([128, c.NB], F32, tag="dinv_col")
            iota_sb = pp.tile([128, 128], BF16, tag="iota")
            eye2_sb = pp.tile([128, 128], BF16, tag="eye2")
            id_sb = pp.tile([128, 128], F32, tag="idf")
            w_sb = [pp.tile([128, 128], F32, tag=f"w{l}", name=f"w{l}_sb") for l in range(3)]
            g_sb = [pp.tile([128, 1], F32, tag=f"g{l}", name=f"g{l}_sb") for l in range(3)]
            be_sb = [pp.tile([128, 1], F32, tag=f"be{l}", name=f"be{l}_sb") for l in range(3)]

            nc.sync.dma_start(h_sb[:], x_t.ap())
            nc.sync.dma_start(ld_sb[:], ld_d.ap())
            for h in (0, 1):
                nc.sync.dma_start(idx_sb[h][:], idx_d[h].ap())
            nc.sync.dma_start(dinv_col[:], dinv_col_d.ap())
            nc.sync.dma_start(dinv_b[:], dinv_b_d.ap())
            nc.sync.dma_start(iota_sb[:], iota_d.ap())
            nc.sync.dma_start(eye2_sb[:], eye2_d.ap())
            nc.sync.dma_start(id_sb[:], id_d.ap())
            for l in range(3):
                nc.sync.dma_start(w_sb[l][:], w_d[l].ap())
                nc.sync.dma_start(g_sb[l][:], g_d[l].ap())
                nc.sync.dma_start(be_sb[l][:], be_d[l].ap())

            u_loc_ap = u_loc.ap().rearrange("(g p) f -> p g f", p=128)

            for l in range(3):
                # ---- u_rows = dinv[n] * (h @ W) in [node, feat] layout ----
                for g in range(c.NB):
                    pu = pup.tile([128, 128], F32, tag="pu")
                    nc.tensor.matmul(pu[:], lhsT=h_sb[:, g * 128:(g + 1) * 128],
                                     rhs=w_sb[l][:], start=True, stop=True)
                    nc.vector.tensor_scalar(u_rows[:, g * 128:(g + 1) * 128], pu[:],
                                            dinv_col[:, g:g + 1], None,
                                            mybir.AluOpType.mult)
                nc.sync.dma_start(u_loc_ap, u_rows[:].rearrange("p (g f) -> p g f", f=128))

                if "ag" in stages:
                    nc.gpsimd.collective_compute(
                        "AllGather", mybir.AluOpType.bypass, replica_groups=rg,
                        ins=[u_loc.ap()[0:c.NPC, :]], outs=[u_full.ap()],
                    )

                # ---- gather message chunks ----
                # Emit gathers in window-consumption order: interleave the two
                # half-streams by the window that first consumes each chunk, so
                # early windows never wait behind the whole other stream.
                msg_tiles = [[], []]
                qn = 0
                if "gather" in stages:
                    def first_window(h, t0):
                        for w in range(c.W):
                            if int(meta["off_tiles"][w, h]) + int(nt[w, h]) > t0:
                                return w
                        return c.W
                    order = sorted(
                        [(h, t0, ctn) for h in (0, 1) for (t0, ctn) in chunks[h]],
                        key=lambda x: (first_window(x[0], x[1]), x[0]))
                    for (h, t0, ctn) in order:
                        src_ap = u_full.ap()[0:c.HALF, :] if h == 0 else u_full.ap()[c.HALF:c.N, :]
                        pool = m0p if h == 0 else m1p
                        mt = pool.tile([128, c.CT, 128], BF16, tag=f"m{h}", name=f"m{h}_chunk")
                        nidx = ctn * 128
                        nc.gpsimd.dma_gather(
                            out_ap=mt[:, 0:ctn, :],
                            in_ap=src_ap,
                            idxs_ap=idx_sb[h][:, t0 * 8:(t0 + ctn) * 8],
                            num_idxs=nidx,
                            num_idxs_reg=nidx,
                            elem_size=128,
                            queue_num=qn % 4,
                        )
                        qn += 1
                        msg_tiles[h].append((t0, ctn, mt))

                def msg_slice(h, gtile):
                    for (t0, ctn, mt) in msg_tiles[h]:
                        if t0 <= gtile < t0 + ctn:
                            return mt[:, gtile - t0, :]
                    raise AssertionError

                # ---- windows: one-hot + matmul accumulate ----
                NTMAX = int(nt.max())
                tcol = 0
                for w in range(c.W if "agg" in stages else 0):
                    pw = pwp.tile([128, 128], F32, tag="pw")
                    first = True
                    for h in (0, 1):
                        ntg = int(nt[w, h])
                        s_g = sp.tile([128, NTMAX, 128], BF16, tag="s", name="s_g")
                        iota_bc = iota_sb[:].rearrange("p (a b) -> p a b", a=1) \
                            .to_broadcast([128, ntg, 128])
                        ld_bc = ld_sb[:, tcol:tcol + ntg] \
                            .rearrange("p (a b) -> p a b", b=1) \
                            .to_broadcast([128, ntg, 128])
                        nc.vector.tensor_tensor(s_g[:, 0:ntg, :], iota_bc, ld_bc,
                                                mybir.AluOpType.is_equal)
                        for k in range(ntg):
                            gtile = int(meta["off_tiles"][w, h]) + k
                            nc.tensor.matmul(pw[:], lhsT=msg_slice(h, gtile),
                                             rhs=s_g[:, k, :],
                                             start=first, stop=False)
                            first = False
                            tcol += 1
                    nc.tensor.matmul(pw[:], lhsT=u_rows[:, w * 128:(w + 1) * 128],
                                     rhs=eye2_sb[:], start=first, stop=True)
                    wd = min(128, c.NPC - w * 128)
                    nc.vector.tensor_mul(agg[:, w * 128:w * 128 + wd], pw[:, 0:wd],
                                         dinv_b[:, w * 128:w * 128 + wd])

                if "agg" not in stages:
                    nc.vector.memset(agg[:], 0.01 * (l + 1))
                if "bn" not in stages:
                    if "bn1" in stages:
                        sum_c = cp.tile([128, 1], F32, tag="sum")
                        ssq_c = cp.tile([128, 1], F32, tag="ssq")
                        nc.vector.tensor_reduce(sum_c[:], agg[:, 0:c.NPC],
                                                axis=mybir.AxisListType.X, op=mybir.AluOpType.add)
                        if "ttr" in stages:
                            nc.vector.tensor_tensor_reduce(
                                h_sb[:, 0:c.NPC], agg[:, 0:c.NPC], agg[:, 0:c.NPC], 1.0, 0.0,
                                mybir.AluOpType.mult, mybir.AluOpType.add, ssq_c[:])
                        else:
                            nc.vector.tensor_reduce(ssq_c[:], agg[:, 0:c.NPC],
                                                    axis=mybir.AxisListType.X, op=mybir.AluOpType.add)
                        st_sb = cp.tile([128, 2], F32, tag="st")
                        nc.vector.tensor_copy(st_sb[:, 0:1], sum_c[:])
                        nc.vector.tensor_copy(st_sb[:, 1:2], ssq_c[:])
                        nc.sync.dma_start(stats_in.ap(), st_sb[:])
                        nc.gpsimd.collective_compute(
                            "AllReduce", mybir.AluOpType.add, replica_groups=rg,
                            ins=[stats_in.ap()], outs=[stats_out.ap()],
                        )
                        st2 = cp.tile([128, 2], F32, tag="st2")
                        nc.sync.dma_start(st2[:], stats_out.ap())
                    nc.scalar.activation(h_sb[:, 0:c.NPC], agg[:, 0:c.NPC],
                                         mybir.ActivationFunctionType.Relu)
                    continue
                # ---- BatchNorm (training stats over all N nodes) + ReLU ----
                sum_c = cp.tile([128, 1], F32, tag="sum")
                ssq_c = cp.tile([128, 1], F32, tag="ssq")
                nc.vector.tensor_reduce(sum_c[:], agg[:, 0:c.NPC],
                                        axis=mybir.AxisListType.X, op=mybir.AluOpType.add)
                nc.vector.tensor_mul(h_sb[:, 0:c.NPC], agg[:, 0:c.NPC], agg[:, 0:c.NPC])
                nc.vector.tensor_reduce(ssq_c[:], h_sb[:, 0:c.NPC],
                                        axis=mybir.AxisListType.X, op=mybir.AluOpType.add)
                st_sb = cp.tile([128, 2], F32, tag="st")
                nc.vector.tensor_copy(st_sb[:, 0:1], sum_c[:])
                nc.vector.tensor_copy(st_sb[:, 1:2], ssq_c[:])
                nc.sync.dma_start(stats_in.ap(), st_sb[:])
                nc.gpsimd.collective_compute(
                    "AllReduce", mybir.AluOpType.add, replica_groups=rg,
                    ins=[stats_in.ap()], outs=[stats_out.ap()],
                )
                st2 = cp.tile([128, 2], F32, tag="st2")
                nc.sync.dma_start(st2[:], stats_out.ap())
                mean_c = cp.tile([128, 1], F32, tag="mean")
                var_c = cp.tile([128, 1], F32, tag="var")
                t1 = cp.tile([128, 1], F32, tag="t1")
                s_col = cp.tile([128, 1], F32, tag="scol")
                t_col = cp.tile([128, 1], F32, tag="tcol")
                inv_n = 1.0 / float(c.N)
                nc.vector.tensor_scalar_mul(mean_c[:], st2[:, 0:1], inv_n)
                nc.vector.tensor_scalar_mul(var_c[:], st2[:, 1:2], inv_n)   # E[x^2]
                nc.vector.tensor_mul(t1[:], mean_c[:], mean_c[:])
                nc.vector.tensor_sub(var_c[:], var_c[:], t1[:])
                nc.vector.tensor_scalar_add(var_c[:], var_c[:], BN_EPS)
                nc.scalar.sqrt(t1[:], var_c[:])
                nc.vector.reciprocal(t1[:], t1[:])                           # rstd
                nc.vector.tensor_mul(s_col[:], g_sb[l][:], t1[:])
                nc.vector.tensor_mul(t_col[:], mean_c[:], s_col[:])
                nc.vector.tensor_sub(t_col[:], be_sb[l][:], t_col[:])
                # h = relu(agg * s + t)
                nc.scalar.activation(h_sb[:, 0:c.NPC], agg[:, 0:c.NPC],
                                     mybir.ActivationFunctionType.Relu,
                                     bias=t_col[:], scale=s_col[:])

            # ---- output: transpose h -> [node, feat] and store ----
            for g in range(c.NB if "out" in stages else 0):
                pt = pup.tile([128, 128], F32, tag="pu")
                nc.tensor.transpose(pt[:], h_sb[:, g * 128:(g + 1) * 128], id_sb[:])
                nc.vector.tensor_copy(agg[:, g * 128:(g + 1) * 128], pt[:])
            nc.sync.dma_start(y_d.ap().rearrange("(g p) f -> p g f", p=128),
                              agg[:].rearrange("p (g f) -> p g f", f=128))

    nc.compile()
    return nc


_CACHE = {}


def _get_program(cfg, meta):
    key = (cfg.N, cfg.E, cfg.NC, cfg.CT, meta["NT"],
           tuple(meta["s_tiles"].tolist()), meta["nt"].tobytes())
    if key not in _CACHE:
        _CACHE[key] = build(cfg, meta)
    return _CACHE[key]


def run(cfg, inputs, trace=False):
    Ws = [inputs[f"W{l}"] for l in range(3)]
    gs = [inputs[f"g{l}"] for l in range(3)]
    bes = [inputs[f"be{l}"] for l in range(3)]
    in_maps, meta = preprocess(cfg, inputs["x"], inputs["edge_index"], Ws, gs, bes)
    nc = _get_program(cfg, meta)
    res = bass_utils.run_bass_kernel_spmd(
        nc, in_maps, core_ids=list(range(cfg.NC)), trace=trace)
    out = np.concatenate([res.results[ci]["y"][:cfg.NPC] for ci in range(cfg.NC)], axis=0)
    return out.astype(np.float32), res


def kernel(**inputs):
    cfg = Cfg()
    out, _ = run(cfg, inputs)
    return out
